# revision 9
# baseline (speedup 1.0000x reference)
"""MoE layer (dense all-experts SwiGLU + router-weighted sum) on 8 TRN2 cores.

Expert-parallel: core e holds expert e's weights (E=8). Every core sees the
full token stream x (shipped pre-transposed as xT [H, N]) and computes
  y_e = softmax(x @ W_router)[:, e] * ((silu(x@Wg_e) * (x@Wu_e)) @ Wd_e)
The host sums the 8 per-expert outputs.

All matmul operands are bf16 (inputs quantized on host, ~1e-3 rel err vs
the 2e-2 gate), PSUM accumulation fp32. bf16 halves SBUF so ALL weights
(Wg, Wu, Wd) are resident: no per-block weight streaming at all. The only
steady-state DMA is xt in (1 MB/block, sync HWDGE ring, 3 blocks deep) and
y out (2 MB/block, riding the otherwise-idle SWDGE ring; the last block
rides sync so the tail drains fast).

Per-core program, per 512-token block:
  router: logits^T [8,512] via PE (Wr stationary, xT moving), Exp on ACT,
          then per 128-token subtile a transpose-matmul with rhs=[ones|e_sel]
          gives [denom | numer] in PSUM -> w = numer * 1/denom on DVE.
  stage1: G/U [128i, 512tok] = Wg/Wu_chunk^T @ xT_chunk (8 K-chunks in PSUM),
          hT[i] = silu(G)*U -> SBUF bf16 (resident for the block, [I, tok]).
  stage2: Y[m] [128tok, 512h] accumulates over 16 i-chunks with hT as
          stationary and resident Wd slices as moving; evict = DVE multiply
          by the router weight, y DMA issued per tile.

Block 0 runs stage 1 k-OUTER in groups of 4 i-chunks (borrowing the psy
PSUM banks) so compute starts as soon as the first 512KB Wg k-chunk lands
(~5us) instead of waiting for the full Wg (~16us). Weight DMAs are issued
k-chunk-granular in consumption order: wr, xt(0), wg, xt(1), wu, wd, xt(2).

Scheduling notes:
  - tile-pool slot allocation order must match consumption order or the
    schedule deadlocks; all 8 xt chunks of 3 blocks are live at once.
  - next block's router runs between the two h-sweeps (PE filler).
"""
import numpy as np

import concourse.bass as bass
import concourse.mybir as mybir
import concourse.tile as tile
from concourse import bacc
from concourse.bass_utils import run_bass_kernel_spmd

P = 128
H, I, E = 1024, 2048, 8
N = 8192  # tokens = 4 * 2048
HK = H // P   # 8 contraction chunks over H
IK = I // P   # 16 chunks over I
TB = 512      # token block
NB = N // TB  # 16 blocks
NM = TB // P  # 4 token subtiles per block
NH = H // 512  # 2 output column halves
PF = 3        # xt prefetch depth (blocks)

F32 = mybir.dt.float32
DT16 = mybir.dt.float16
AF = mybir.ActivationFunctionType

# set by a driver (test.py) to profile; harness path keeps defaults
TRACE = False
LAST_EXEC_NS = None

_CACHE = {}


def _build():
    nc = bacc.Bacc("TRN2", target_bir_lowering=False, debug=False)

    xt_d = nc.dram_tensor("xt", [H, N], DT16, kind="ExternalInput").ap()
    wg_d = nc.dram_tensor("wg", [H, I], DT16, kind="ExternalInput").ap()
    wu_d = nc.dram_tensor("wu", [H, I], DT16, kind="ExternalInput").ap()
    wd_d = nc.dram_tensor("wd", [I, H], DT16, kind="ExternalInput").ap()
    wr_d = nc.dram_tensor("wr", [P, HK * E], DT16, kind="ExternalInput").ap()
    sel_d = nc.dram_tensor("sel", [E, 2], DT16, kind="ExternalInput").ap()
    y_d = nc.dram_tensor("y", [N, H], DT16, kind="ExternalOutput").ap()

    with tile.TileContext(nc) as tc:
        with (
            tc.tile_pool(name="const", bufs=1) as const,
            tc.tile_pool(name="xtp", bufs=PF * HK) as xtp,
            tc.tile_pool(name="htp", bufs=1) as htp,
            tc.tile_pool(name="evp", bufs=8) as evp,
            tc.tile_pool(name="rtp", bufs=2) as rtp,
            tc.tile_pool(name="wp", bufs=2) as wp,
            tc.tile_pool(name="psgu", bufs=2, space="PSUM") as psgu,
            tc.tile_pool(name="psy", bufs=5, space="PSUM") as psy,
            tc.tile_pool(name="psr", bufs=1, space="PSUM") as psr,
        ):
            # resident weights: [128, HK*I] with chunk k at cols [k*I, (k+1)*I)
            wg_sb = const.tile([P, HK * I], DT16)
            wu_sb = const.tile([P, HK * I], DT16)
            # wd resident: [128, IK*H] with i-chunk at cols [i*H, (i+1)*H)
            wd_sb = const.tile([P, IK * H], DT16)
            wr_sb = const.tile([P, HK * E], DT16)
            sel_sb = const.tile([E, 2], DT16)

            def load_xt(b):
                tok = slice(b * TB, (b + 1) * TB)
                chunks = []
                for k in range(HK):
                    ch = xtp.tile([P, TB], DT16, tag="xt", name=f"xt{b}_{k}")
                    # ACT hwdge ring: xt streams in parallel with the weight
                    # stream on the SP ring (prologue) / keeps SP free.
                    nc.scalar.dma_start(
                        out=ch[:], in_=xt_d[k * P:(k + 1) * P, tok]
                    )
                    chunks.append(ch)
                return chunks

            # ---- prologue DMAs in consumption order (sync ring is FIFO)
            nc.sync.dma_start(out=wr_sb[:], in_=wr_d[:])
            xt_next = load_xt(0)
            nc.sync.dma_start(out=sel_sb[:], in_=sel_d[:])
            IH = I // 2
            # wg/wu streamed as 256KB half-chunks, half-A (i 0-7) of every
            # k first: block-0's k-outer groups consume them as they land.
            for half in range(2):
                for k in range(HK):
                    nc.sync.dma_start(
                        out=wg_sb[:, k * I + half * IH: k * I + (half + 1) * IH],
                        in_=wg_d[k * P:(k + 1) * P, half * IH:(half + 1) * IH],
                    )
            xt_pre1 = load_xt(1)
            for half in range(2):
                for k in range(HK):
                    nc.sync.dma_start(
                        out=wu_sb[:, k * I + half * IH: k * I + (half + 1) * IH],
                        in_=wu_d[k * P:(k + 1) * P, half * IH:(half + 1) * IH],
                    )
            for j in range(IK // 2):  # wd as 512KB 2-chunk batches
                rows = slice(2 * j * P, (2 * j + 2) * P)
                nc.sync.dma_start(
                    out=wd_sb[:, 2 * j * H:(2 * j + 2) * H].rearrange(
                        "p (j c) -> p j c", j=2),
                    in_=wd_d[rows, :].rearrange("(j p) c -> p j c", p=P),
                )

            def router(xt_ch):
                # w[tok] = softmax(logits)[:, e] for one block
                lt = psr.tile([E, TB], F32, tag="rt", name="lt")
                for k in range(HK):
                    nc.tensor.matmul(
                        lt[:],
                        (wr_sb[:, k * E:(k + 1) * E]),
                        (xt_ch[k][:]),
                        start=(k == 0),
                        stop=(k == HK - 1),
                    )
                exp_sb = rtp.tile([E, TB], DT16, tag="exp", name="exp_sb")
                nc.scalar.activation(exp_sb[:], lt[:], AF.Exp)
                w_tiles = []
                for m in range(NM):
                    dn = psr.tile([P, 2], F32, tag="rt", name="dn")
                    nc.tensor.matmul(
                        dn[:],
                        (exp_sb[:, m * P:(m + 1) * P]),
                        (sel_sb[:]),
                        start=True,
                        stop=True,
                    )
                    rec = wp.tile([P, 1], F32, tag="rec", name="rec")
                    nc.vector.reciprocal(rec[:], dn[:, 0:1])
                    w_m = wp.tile([P, 1], F32, tag=f"w{m}", name="w_m")
                    nc.vector.tensor_tensor(
                        out=w_m[:], in0=dn[:, 1:2], in1=rec[:],
                        op=mybir.AluOpType.mult,
                    )
                    w_tiles.append(w_m)
                return w_tiles

            xtq = [xt_next, xt_pre1]  # blocks b, b+1 (already issued)
            w_next = router(xtq[0])
            for b in range(NB):
                xt_ch = xtq.pop(0)
                w_tiles = w_next
                if b + 2 < NB:
                    xtq.append(load_xt(b + 2))

                # ---- stage 1: hT[i] = silu(G)*U, [I-chunk, tok] layout
                ht_sb = htp.tile([P, IK * TB], DT16, tag="ht")

                def g_step(i):
                    g_ps = psgu.tile([P, TB], F32, tag="gu", name="g_ps")
                    for k in range(HK):
                        nc.tensor.matmul(
                            g_ps[:],
                            (wg_sb[:, k * I + i * P: k * I + (i + 1) * P]),
                            (xt_ch[k][:]),
                            start=(k == 0),
                            stop=(k == HK - 1),
                        )
                    nc.scalar.activation(
                        ht_sb[:, i * TB:(i + 1) * TB], g_ps[:], AF.Silu
                    )

                def u_step(i):
                    u_ps = psgu.tile([P, TB], F32, tag="gu", name="u_ps")
                    for k in range(HK):
                        nc.tensor.matmul(
                            u_ps[:],
                            (wu_sb[:, k * I + i * P: k * I + (i + 1) * P]),
                            (xt_ch[k][:]),
                            start=(k == 0),
                            stop=(k == HK - 1),
                        )
                    hsl = ht_sb[:, i * TB:(i + 1) * TB]
                    nc.vector.tensor_tensor(
                        out=hsl, in0=hsl, in1=u_ps[:], op=mybir.AluOpType.mult
                    )

                if b == 0:
                    # k-outer in groups of 4 i-chunks (borrowing psy banks):
                    # each 512KB wg k-chunk unlocks 4 matmuls as it lands.
                    for w_sb, is_g in ((wg_sb, True), (wu_sb, False)):
                        for grp in range(IK // 4):
                            ps4 = [
                                psy.tile([P, TB], F32, tag="y",
                                         name=f"b0_{'g' if is_g else 'u'}{grp}_{j}")
                                for j in range(4)
                            ]
                            for k in range(HK):
                                for j in range(4):
                                    i = grp * 4 + j
                                    nc.tensor.matmul(
                                        ps4[j][:],
                                        (w_sb[:, k * I + i * P:
                                              k * I + (i + 1) * P]),
                                        (xt_ch[k][:]),
                                        start=(k == 0),
                                        stop=(k == HK - 1),
                                    )
                            for j in range(4):
                                i = grp * 4 + j
                                hsl = ht_sb[:, i * TB:(i + 1) * TB]
                                if is_g:
                                    nc.scalar.activation(hsl, ps4[j][:], AF.Silu)
                                else:
                                    nc.vector.tensor_tensor(
                                        out=hsl, in0=hsl, in1=ps4[j][:],
                                        op=mybir.AluOpType.mult,
                                    )
                else:
                    for i in range(IK):
                        g_step(i)
                        u_step(i)

                # ---- stage 2: Y[m] [128tok, 512h] = hT^T @ Wd (resident),
                # scaled by w on evict; y DMA per tile on the idle SWDGE ring.
                if b + 1 < NB:
                    for h in range(NH):
                        if h == 1:
                            # next block's router between the h-sweeps
                            w_next = router(xtq[0])
                        y_ps = [
                            psy.tile([P, 512], F32, tag="y", name=f"y_ps{m}")
                            for m in range(NM)
                        ]
                        for i in range(IK):
                            rhs = wd_sb[:, i * H + h * 512:
                                        i * H + (h + 1) * 512]
                            for m in range(NM):
                                nc.tensor.matmul(
                                    y_ps[m][:],
                                    (ht_sb[:, i * TB + m * P:
                                           i * TB + (m + 1) * P]),
                                    (rhs),
                                    start=(i == 0),
                                    stop=(i == IK - 1),
                                )
                        for m in range(NM):
                            y_sb = evp.tile([P, 512], DT16, tag="ev",
                                            name=f"yev{h}_{m}")
                            nc.vector.tensor_scalar_mul(
                                y_sb[:], y_ps[m][:], w_tiles[m][:]
                            )
                            nc.gpsimd.dma_start(
                                out=y_d[b * TB + m * P: b * TB + (m + 1) * P,
                                        h * 512:(h + 1) * 512],
                                in_=y_sb[:],
                            )
                else:
                    # last block: m-outer so earlier m's evict+DMA (sync
                    # ring, now idle) overlap later m's matmuls -> short tail
                    for m in range(NM):
                        y2 = [
                            psy.tile([P, 512], F32, tag="y", name=f"yl{m}_{h}")
                            for h in range(NH)
                        ]
                        for i in range(IK):
                            st = ht_sb[:, i * TB + m * P: i * TB + (m + 1) * P]
                            for h in range(NH):
                                nc.tensor.matmul(
                                    y2[h][:],
                                    (st),
                                    (wd_sb[:, i * H + h * 512:
                                           i * H + (h + 1) * 512]),
                                    start=(i == 0),
                                    stop=(i == IK - 1),
                                )
                        for h in range(NH):
                            y_sb = evp.tile([P, 512], DT16, tag="ev",
                                            name=f"yevl{m}_{h}")
                            nc.vector.tensor_scalar_mul(
                                y_sb[:], y2[h][:], w_tiles[m][:]
                            )
                            nc.sync.dma_start(
                                out=y_d[b * TB + m * P: b * TB + (m + 1) * P,
                                        h * 512:(h + 1) * 512],
                                in_=y_sb[:],
                            )

    nc.compile()
    return nc


def kernel(x, W_router, W_gate, W_up, W_down):
    global LAST_EXEC_NS
    if "nc" not in _CACHE:
        _CACHE["nc"] = _build()
    nc = _CACHE["nc"]

    bf16 = np.float16
    x_bf = np.asarray(x, dtype=np.float32).reshape(N, H).astype(bf16)
    xt = np.ascontiguousarray(x_bf.T)
    # repack router weights into the SBUF layout [128, HK*E]: row p holds
    # chunk k's rows (k*128+p) side by side -> plain contiguous DMA on device
    wr = np.ascontiguousarray(
        np.asarray(W_router, dtype=np.float32)
        .reshape(HK, P, E).transpose(1, 0, 2).reshape(P, HK * E)
    ).astype(bf16)
    eye = np.eye(E, dtype=np.float32)
    in_maps = []
    for e in range(E):
        sel = np.stack([np.ones(E, dtype=np.float32), eye[e]], axis=1)
        in_maps.append({
            "xt": xt,
            "wg": np.ascontiguousarray(W_gate[e]).astype(bf16),
            "wu": np.ascontiguousarray(W_up[e]).astype(bf16),
            "wd": np.ascontiguousarray(W_down[e]).astype(bf16),
            "wr": wr,
            "sel": np.ascontiguousarray(sel).astype(bf16),
        })

    res = run_bass_kernel_spmd(nc, in_maps, list(range(E)), trace=TRACE)
    LAST_EXEC_NS = res.exec_time_ns

    acc = np.zeros((N, H), dtype=np.float64)
    for r in res.results:
        acc += r["y"]
    return acc.astype(np.float32).reshape(x.shape[0], x.shape[1], H)
